# revision 16
# baseline (speedup 1.0000x reference)
"""Causal self-attention on 8 TRN2 NeuronCores (Bass/Tile, SPMD).

Problem: y = CausalSelfAttention(x; Wqkv, bqkv, Wproj, bproj)
  x [B=4, T=2048, C=1024], H=16 heads, D=64.

Sharding: core c = (batch b = c//2, head-half hh = c%2). Each core computes
q/k/v for its 8 heads of its batch (Wqkv column-sharded), full causal
attention for those heads, and a partial output projection (Wproj
row-sharded). Host sums the two bf16 partials per batch and adds bproj.

Per-core kernel (all matmuls bf16 with fp32 PSUM accumulation):
  - q,k are produced d-major ([CL, T]) so QK^T needs no transposes;
    scores come out k-major [128 k, 512 q] per tile. The two heads of a
    pair run as concurrent 64x128 row-tiles of the PE array.
  - softmax skips the max-subtraction (scores are O(1); exp is safe),
    one fused exp on the Scalar engine per k-tile; causal mask is a bf16
    multiply on the diagonal blocks only. Row sums come free from an
    extra ones-column appended to each per-head V tile (M=65 AV matmul);
    1/sum is broadcast across partitions via a DRAM round-trip DMA.
  - Sub-diagonal k-tiles are skipped entirely (half the attention work).

Scheduling (the attention exp stream saturates the Scalar engine while
the PE has slack; everything else is interleaved into that slack):
  - v-projection tiles, q/k projections for later head-pairs, softmax
    normalization, and 3/4 of the output projection are emitted as
    fine-grained filler units between attention k-tile groups, so the
    PE and ACT engines both stay busy from prologue to tail.
  - k-tiles are processed in groups of two (scores+exp for both, then
    the four staggered AV matmuls) to halve PE tiling-mode switches.
  - normalization runs per (head-pair, q-block) as soon as that
    q-block's denominators land; the output projection for a T-slice
    starts as soon as the last head-pair has normalized it.
"""

import math
from contextlib import ExitStack

import numpy as np
import ml_dtypes

import concourse.tile as tile
from concourse import bacc, mybir

BF16 = mybir.dt.bfloat16
F32 = mybir.dt.float32
NPBF16 = ml_dtypes.bfloat16

P = 128  # partitions / k-tile size
QB = 512  # q-block (matmul N; one fp32 PSUM bank)

B, T, C, H, D = 4, 2048, 1024, 16, 64
N_CORES = 8
HL = H // (N_CORES // B)  # heads per core (8)
CL = HL * D  # local head width (512)

# ---------------------------------------------------------------------------
# Per-core Bass program
# ---------------------------------------------------------------------------


def build_kernel(T=T, C=C, HL=HL, D=D, Cout=C):
    CL = HL * D
    n_ct = C // P
    n_mt = CL // P
    n_tt = T // P
    n_qb = T // QB
    n_hp = HL // 2
    dpb = QB // P
    n_cb = Cout // QB
    scale = 1.0 / math.sqrt(D)
    D1 = D + 1
    n_sums = n_hp * n_qb * 2  # one softmax-denominator row per (head, q-block)

    assert C % P == 0 and CL % P == 0 and T % QB == 0 and Cout % QB == 0
    assert HL % 2 == 0 and D == 64 and n_mt == n_hp and n_sums <= P
    assert n_qb == 4 and n_hp == 4 and n_tt == 16  # schedule table below

    nc = bacc.Bacc("TRN2", target_bir_lowering=False, debug=False)
    xT = nc.dram_tensor("xT", [C, T], BF16, kind="ExternalInput")
    wq = nc.dram_tensor("wq", [C, CL], BF16, kind="ExternalInput")
    wk = nc.dram_tensor("wk", [C, CL], BF16, kind="ExternalInput")
    wv = nc.dram_tensor("wv", [C, CL], BF16, kind="ExternalInput")
    wp = nc.dram_tensor("wp", [CL, Cout], BF16, kind="ExternalInput")
    masks = nc.dram_tensor("masks", [P, P], BF16, kind="ExternalInput")
    out = nc.dram_tensor("out", [T, Cout], BF16, kind="ExternalOutput")

    with tile.TileContext(nc) as tc, ExitStack() as ctx:
        persist = ctx.enter_context(tc.tile_pool(name="persist", bufs=1))
        # PSUM budget (8 banks): st 2 x [128,1024] (4) + y 2 x [65,512] (2)
        # + u512 2 x [128,512] (2)
        ps_st = ctx.enter_context(tc.tile_pool(name="ps_st", bufs=2, space="PSUM"))
        ps_y = ctx.enter_context(tc.tile_pool(name="ps_y", bufs=2, space="PSUM"))
        ps_u512 = ctx.enter_context(tc.tile_pool(name="ps_u512", bufs=2, space="PSUM"))
        ppool = ctx.enter_context(tc.tile_pool(name="ppool", bufs=6))
        spool = ctx.enter_context(tc.tile_pool(name="spool", bufs=4))
        bcpool = ctx.enter_context(tc.tile_pool(name="bcpool", bufs=4))
        stage = ctx.enter_context(tc.tile_pool(name="stage", bufs=4))
        dram = ctx.enter_context(tc.tile_pool(name="dram", bufs=1, space="DRAM"))

        # ---- persistent SBUF tensors (consolidated so each input loads
        # with one big DMA instead of one per 128-row tile) ----
        xT_all = persist.tile([P, n_ct * T], BF16, tag="xT", name="xT")
        wv_all = persist.tile([P, n_ct * CL], BF16, tag="wv", name="wv")
        wq_all = persist.tile([P, n_ct * CL], BF16, tag="wq", name="wq")
        wk_all = persist.tile([P, n_ct * CL], BF16, tag="wk", name="wk")
        wp_all = persist.tile([P, n_mt * Cout], BF16, tag="wp", name="wp")
        xT_sb = [xT_all[:, i * T : (i + 1) * T] for i in range(n_ct)]
        wv_sb = [wv_all[:, i * CL : (i + 1) * CL] for i in range(n_ct)]
        wq_sb = [wq_all[:, i * CL : (i + 1) * CL] for i in range(n_ct)]
        wk_sb = [wk_all[:, i * CL : (i + 1) * CL] for i in range(n_ct)]
        wp_sb = [wp_all[:, i * Cout : (i + 1) * Cout] for i in range(n_mt)]
        trimask = persist.tile([P, P], BF16, tag="trimask", name="trimask")
        v1_sb = [
            persist.tile([P, HL * D1], BF16, tag=f"v1_{tt}", name=f"v1_{tt}")
            for tt in range(n_tt)
        ]
        yT_sb = [
            persist.tile([P, T], BF16, tag=f"yT{m}", name=f"yT{m}")
            for m in range(n_mt)
        ]
        q_d = [[None] * n_qb for _ in range(n_hp)]
        k_d = [[None] * n_qb for _ in range(n_hp)]

        sums_d = dram.tile([n_sums, QB], F32, tag="sums_d", name="sums_d")
        recips_d = dram.tile([n_sums, QB], F32, tag="recips_d", name="recips_d")

        # ---- input DMA: one instruction per tensor/chunk (queue-trigger
        # instructions cost ~650ns each, so batching matters). First chunk
        # of xT + wv (for v0..3 and qk(0,0)), then wq/wk, then the rest of
        # xT in two chunks, wp last ----
        T0 = 4 * P  # columns needed by the prologue (v0..3, qk(0,0))
        T1 = 10 * P
        xT_src = xT[:].rearrange("(c p) t -> p c t", p=P)
        xT_dst = xT_all[:].rearrange("p (c t) -> p c t", t=T)
        w_srcs = {
            "wv": wv[:].rearrange("(c p) t -> p c t", p=P),
            "wq": wq[:].rearrange("(c p) t -> p c t", p=P),
            "wk": wk[:].rearrange("(c p) t -> p c t", p=P),
            "wp": wp[:].rearrange("(c p) t -> p c t", p=P),
        }
        w_dsts = {
            "wv": wv_all[:].rearrange("p (c t) -> p c t", t=CL),
            "wq": wq_all[:].rearrange("p (c t) -> p c t", t=CL),
            "wk": wk_all[:].rearrange("p (c t) -> p c t", t=CL),
            "wp": wp_all[:].rearrange("p (c t) -> p c t", t=Cout),
        }
        nc.sync.dma_start(xT_dst[:, :, 0:T0], xT_src[:, :, 0:T0])
        nc.sync.dma_start(w_dsts["wv"], w_srcs["wv"])
        nc.sync.dma_start(trimask[:], masks[:])
        nc.sync.dma_start(w_dsts["wq"], w_srcs["wq"])
        nc.sync.dma_start(w_dsts["wk"], w_srcs["wk"])
        nc.sync.dma_start(xT_dst[:, :, T0:T1], xT_src[:, :, T0:T1])
        nc.sync.dma_start(xT_dst[:, :, T1:T], xT_src[:, :, T1:T])
        nc.sync.dma_start(w_dsts["wp"], w_srcs["wp"])

        # ---- filler units (one PE burst each, ~0.7-1.7us) ----

        def v_unit(tt):
            t = v1_sb[tt]
            ones_view = t[:].rearrange("p (h e) -> p h e", h=HL)[:, :, D : D + 1]
            nc.vector.memset(ones_view, 1.0)
            ps = ps_u512.tile([P, CL], F32, tag="u512", name="u512")
            for c in range(n_ct):
                nc.tensor.matmul(
                    ps[:],
                    xT_sb[c][:, tt * P : (tt + 1) * P],
                    wv_sb[c][:],
                    start=(c == 0),
                    stop=(c == n_ct - 1),
                )
            dst_view = t[:].rearrange("p (h e) -> p h e", h=HL)[:, :, 0:D]
            src_view = ps[:].rearrange("p (h e) -> p h e", h=HL)
            nc.vector.tensor_copy(dst_view, src_view)

        def qk_unit(hp, b, name):
            w_sb, dst = (wk_sb, k_d) if name == "k" else (wq_sb, q_d)
            t = persist.tile(
                [P, QB], BF16, tag=f"{name}d{hp}_{b}", name=f"{name}d{hp}_{b}"
            )
            dst[hp][b] = t
            ps = ps_u512.tile([P, QB], F32, tag="u512", name="u512")
            for c in range(n_ct):
                nc.tensor.matmul(
                    ps[:],
                    w_sb[c][:, hp * P : (hp + 1) * P],
                    xT_sb[c][:, b * QB : (b + 1) * QB],
                    start=(c == 0),
                    stop=(c == n_ct - 1),
                )
            nc.vector.tensor_copy(t[:], ps[:])

        def proj_unit(tt, cb):
            ps = ps_u512.tile([P, QB], F32, tag="u512", name="u512")
            for m in range(n_mt):
                nc.tensor.matmul(
                    ps[:],
                    yT_sb[m][:, tt * P : (tt + 1) * P],
                    wp_sb[m][:, cb * QB : (cb + 1) * QB],
                    start=(m == 0),
                    stop=(m == n_mt - 1),
                )
            ost = stage.tile([P, QB], BF16, tag="ostage", name="ostage")
            nc.vector.tensor_copy(ost[:], ps[:])
            nc.sync.dma_start(
                out[tt * P : (tt + 1) * P, cb * QB : (cb + 1) * QB], ost[:]
            )

        def sum_row(hp, qb, i):
            return (hp * n_qb + qb) * 2 + i

        def norm_unit(hp, qb):
            # reciprocal of this q-block's denominators, broadcast across
            # partitions via a DRAM round trip, then scale yT in place.
            s0 = sum_row(hp, qb, 0)
            allsums = stage.tile([2, QB], F32, tag="allsums", name="allsums")
            nc.sync.dma_start(allsums[:], sums_d[s0 : s0 + 2, :])
            allrec = stage.tile([2, QB], F32, tag="allrec", name="allrec")
            nc.vector.reciprocal_approx_fast(allrec[:], allsums[:])
            nc.sync.dma_start(recips_d[s0 : s0 + 2, :], allrec[:])
            bc = bcpool.tile([P, QB], F32, tag="bc", name="bc")
            for i in range(2):
                s = sum_row(hp, qb, i)
                nc.sync.dma_start(
                    bc[64 * i : 64 * i + 64, :],
                    recips_d[s : s + 1, :].to_broadcast((64, QB)),
                )
            sl = yT_sb[hp][:, qb * QB : (qb + 1) * QB]
            nc.vector.tensor_mul(sl, sl, bc[:])

        # ---- attention for one (head-pair, q-block): generator that
        # yields once per 2-k-tile group so the scheduler can emit filler
        # PE work into the ACT-paced slack ----

        def attn_qb(hp, qb):
            yts = [
                ps_y.tile([D1, QB], F32, tag="yt", name="yt0"),
                ps_y.tile([D1, QB], F32, tag="yt", name="yt1"),
            ]
            n_kt = dpb * qb + dpb

            def emit_av(kt, pt):
                # diagonal k-tiles only touch q-columns >= P*m
                q0 = P * max(kt - dpb * qb, 0)
                for i in range(2):
                    h = 2 * hp + i
                    nc.tensor.matmul(
                        yts[i][:, q0:QB],
                        v1_sb[kt][:, h * D1 : (h + 1) * D1],
                        pt[:, i * QB + q0 : (i + 1) * QB],
                        start=(kt == 0),
                        stop=(kt == n_kt - 1),
                        skip_group_check=True,
                    )

            pending = []
            for g0 in range(0, n_kt, 2):
                for kt in (g0, g0 + 1):
                    m = kt - dpb * qb  # >=0: diagonal tile index
                    s0 = P * max(m, 0)
                    # combined scores for both heads: [128 k, 1024];
                    # the two matmuls run as concurrent 64x128 row-tiles
                    st = ps_st.tile([P, 2 * QB], F32, tag="st", name="st")
                    for i in range(2):
                        base = 64 * i
                        nc.tensor.matmul(
                            st[:, i * QB + s0 : (i + 1) * QB],
                            k_d[hp][kt // dpb][
                                base : base + 64,
                                (kt % dpb) * P : (kt % dpb + 1) * P,
                            ],
                            q_d[hp][qb][base : base + 64, s0:],
                            start=True,
                            stop=True,
                        )
                    pt = ppool.tile([P, 2 * QB], BF16, tag="pt", name="pt")
                    if m <= 0:
                        nc.scalar.activation(
                            pt[:],
                            st[:],
                            mybir.ActivationFunctionType.Exp,
                            scale=scale,
                        )
                    else:
                        nc.scalar.activation(
                            pt[:].rearrange("p (i q) -> p i q", i=2)[:, :, s0:],
                            st[:].rearrange("p (i q) -> p i q", i=2)[:, :, s0:],
                            mybir.ActivationFunctionType.Exp,
                            scale=scale,
                        )
                    if m >= 0:
                        q0 = P * m
                        sl = pt[:].rearrange("p (i q) -> p i q", i=2)[
                            :, :, q0 : q0 + P
                        ]
                        nc.vector.tensor_mul(
                            sl, sl, trimask[:, None, :].broadcast_to([P, 2, P])
                        )
                    pending.append((kt, pt))
                # AVs lag the scores by one group so the PE never
                # queue-blocks on exp; one mode switch per direction/group.
                while len(pending) > 2:
                    emit_av(*pending.pop(0))
                yield
            for item in pending:
                emit_av(*item)

            # epilogue: one PSUM->SBUF copy per head, then DMA out the
            # unnormalized y (casting SWDGE) and the denominator row
            for i in range(2):
                yt = yts[i]
                ys = spool.tile([D1, QB], F32, tag="ys", name="ys")
                nc.vector.tensor_copy(ys[:], yt[:])
                nc.gpsimd.dma_start(
                    yT_sb[hp][64 * i : 64 * i + 64, qb * QB : (qb + 1) * QB],
                    ys[0:D, :],
                )
                s = sum_row(hp, qb, i)
                nc.sync.dma_start(sums_d[s : s + 1, :], ys[D : D + 1, :])
            yield

        # ---- global schedule ----
        # Filler units per (hp, qb), distributed across that q-block's
        # yield points. Dependencies: scores(hp,qb) need qk(hp,<=qb);
        # AV(kt) needs v(kt); proj(tt of qb) needs all norms for qb.
        fillers = {
            (0, 0): [("qk", 0, 1, "k"), ("qk", 0, 1, "q"), ("v", 4)],
            (0, 1): [("v", 5), ("v", 6), ("v", 7), ("qk", 0, 2, "k"), ("qk", 0, 2, "q")],
            (0, 2): [("v", 8), ("v", 9), ("v", 10), ("v", 11),
                     ("qk", 0, 3, "k"), ("qk", 0, 3, "q")],
            (0, 3): [("v", 12), ("v", 13), ("v", 14), ("v", 15),
                     ("qk", 1, 0, "k"), ("qk", 1, 0, "q")],
            (1, 0): [("qk", 1, 1, "k"), ("qk", 1, 1, "q")],
            (1, 1): [("qk", 1, 2, "k"), ("qk", 1, 2, "q")],
            (1, 2): [("qk", 1, 3, "k"), ("qk", 1, 3, "q")],
            (1, 3): [("qk", 2, 0, "k"), ("qk", 2, 0, "q")],
            (2, 0): [("qk", 2, 1, "k"), ("qk", 2, 1, "q")],
            (2, 1): [("qk", 2, 2, "k"), ("qk", 2, 2, "q")],
            (2, 2): [("qk", 2, 3, "k"), ("qk", 2, 3, "q")],
            (2, 3): [("qk", 3, 0, "k"), ("qk", 3, 0, "q"),
                     ("qk", 3, 1, "k"), ("qk", 3, 1, "q")],
            (3, 0): [("qk", 3, 2, "k"), ("qk", 3, 2, "q"),
                     ("qk", 3, 3, "k"), ("qk", 3, 3, "q")],
            (3, 1): [("proj", tt, cb) for tt in range(0, 4) for cb in range(n_cb)],
            (3, 2): [("proj", tt, cb) for tt in range(4, 8) for cb in range(n_cb)],
            (3, 3): [("proj", tt, cb) for tt in range(8, 12) for cb in range(n_cb)],
        }

        def emit_filler(u):
            if u[0] == "v":
                v_unit(u[1])
            elif u[0] == "qk":
                qk_unit(u[1], u[2], u[3])
            else:
                proj_unit(u[1], u[2])

        # prologue: enough v-tiles and q/k for hp0's first q-block
        for tt in range(4):
            v_unit(tt)
        qk_unit(0, 0, "k")
        qk_unit(0, 0, "q")

        for hp in range(n_hp):
            for qb in range(n_qb):
                gen = attn_qb(hp, qb)
                units = list(fillers[(hp, qb)])
                n_groups = (dpb * qb + dpb) // 2 + 1  # yield points
                # round-robin the units across yield points
                sched = [[] for _ in range(n_groups)]
                for idx, u in enumerate(units):
                    sched[idx % n_groups].append(u)
                gi = 0
                for _ in gen:
                    # norm for the previous q-block right after the first
                    # group of this one (its sums have landed by then)
                    if gi == 0:
                        if qb > 0:
                            norm_unit(hp, qb - 1)
                        elif hp > 0:
                            norm_unit(hp - 1, n_qb - 1)
                    for u in sched[gi]:
                        emit_filler(u)
                    gi += 1

        # tail: last normalization + the last quarter of the projection
        norm_unit(n_hp - 1, n_qb - 1)
        for tt in range(12, 16):
            for cb in range(n_cb):
                proj_unit(tt, cb)

    nc.compile()
    return nc


_PROGRAM_CACHE = {}


def _get_program(C_eff):
    key = C_eff
    if key not in _PROGRAM_CACHE:
        _PROGRAM_CACHE[key] = build_kernel(T=T, C=C_eff, HL=HL, D=D, Cout=C)
    return _PROGRAM_CACHE[key]


def _make_in_maps(x, Wqkv, bqkv):
    """Shard + cast inputs for the 8 cores. Returns (in_maps, C_eff)."""
    if np.any(bqkv):
        # Fold the qkv bias in as an extra contraction row (x gains a ones
        # column), zero-padded up to a multiple of 128.
        C_eff = ((C + 1 + P - 1) // P) * P
        Waug = np.zeros((C_eff, 3 * C), dtype=np.float32)
        Waug[:C] = Wqkv
        Waug[C] = bqkv
    else:
        C_eff = C
        Waug = Wqkv

    masks = (np.arange(P)[:, None] <= np.arange(P)[None, :]).astype(NPBF16)
    in_maps = []
    for core in range(N_CORES):
        b, hh = divmod(core, N_CORES // B)
        xT = np.zeros((C_eff, T), dtype=np.float32)
        xT[:C] = x[b].T
        if C_eff > C:
            xT[C] = 1.0
        c0 = hh * CL
        in_maps.append(
            {
                "xT": xT.astype(NPBF16),
                "wq": np.ascontiguousarray(Waug[:, 0 * C + c0 : 0 * C + c0 + CL]).astype(NPBF16),
                "wk": np.ascontiguousarray(Waug[:, 1 * C + c0 : 1 * C + c0 + CL]).astype(NPBF16),
                "wv": np.ascontiguousarray(Waug[:, 2 * C + c0 : 2 * C + c0 + CL]).astype(NPBF16),
                "wp": None,  # filled below (depends only on hh)
                "masks": masks,
            }
        )
    return in_maps, C_eff


def _run(x, Wqkv, bqkv, Wproj, bproj, trace=False):
    from concourse.bass_utils import run_bass_kernel_spmd

    in_maps, C_eff = _make_in_maps(x, Wqkv, bqkv)
    wp_by_hh = [
        np.ascontiguousarray(Wproj[hh * CL : (hh + 1) * CL, :]).astype(NPBF16)
        for hh in range(N_CORES // B)
    ]
    for core in range(N_CORES):
        in_maps[core]["wp"] = wp_by_hh[core % (N_CORES // B)]

    nc = _get_program(C_eff)
    res = run_bass_kernel_spmd(
        nc, in_maps, core_ids=list(range(N_CORES)), trace=trace
    )

    halves = N_CORES // B
    y = np.empty((B, T, C), dtype=np.float32)
    for b in range(B):
        acc = res.results[b * halves]["out"].astype(np.float32)
        for hh in range(1, halves):
            acc = acc + res.results[b * halves + hh]["out"].astype(np.float32)
        y[b] = acc + bproj.astype(np.float32)
    return y, res


def kernel(x, Wqkv, bqkv, Wproj, bproj):
    y, _ = _run(
        np.asarray(x, dtype=np.float32),
        np.asarray(Wqkv, dtype=np.float32),
        np.asarray(bqkv, dtype=np.float32),
        np.asarray(Wproj, dtype=np.float32),
        np.asarray(bproj, dtype=np.float32),
        trace=False,
    )
    return y


# revision 21
# speedup vs baseline: 1.0171x; 1.0171x over previous
"""Causal self-attention on 8 TRN2 NeuronCores (Bass/Tile, SPMD).

Problem: y = CausalSelfAttention(x; Wqkv, bqkv, Wproj, bproj)
  x [B=4, T=2048, C=1024], H=16 heads, D=64.

Sharding: core c = (batch b = c//2, head-half hh = c%2). Each core computes
q/k/v for its 8 heads of its batch (Wqkv column-sharded), full causal
attention for those heads, and a partial output projection (Wproj
row-sharded). Host sums the two bf16 partials per batch and adds bproj.

Per-core kernel (all matmuls bf16 with fp32 PSUM accumulation):
  - q,k are produced d-major ([CL, T]) so QK^T needs no transposes;
    scores come out k-major [128 k, 512 q] per tile. The two heads of a
    pair run as concurrent 64x128 row-tiles of the PE array.
  - softmax skips the max-subtraction (scores are O(1); exp is safe),
    one fused exp on the Scalar engine per k-tile; causal mask is a bf16
    multiply on the diagonal blocks only. Row sums come free from an
    extra ones-column appended to each per-head V tile (M=65 AV matmul);
    1/sum is broadcast across partitions via a DRAM round-trip DMA.
  - Sub-diagonal k-tiles are skipped entirely (half the attention work).

Scheduling (the attention exp stream saturates the Scalar engine while
the PE has slack; everything else is interleaved into that slack):
  - v-projection tiles, q/k projections for later head-pairs, softmax
    normalization, and 3/4 of the output projection are emitted as
    fine-grained filler units between attention k-tile groups, so the
    PE and ACT engines both stay busy from prologue to tail.
  - k-tiles are processed in groups of two (scores+exp for both, then
    the four staggered AV matmuls) to halve PE tiling-mode switches.
  - normalization runs per (head-pair, q-block) as soon as that
    q-block's denominators land; the output projection for a T-slice
    starts as soon as the last head-pair has normalized it.
"""

import math
from contextlib import ExitStack

import numpy as np
import ml_dtypes

import concourse.tile as tile
from concourse import bacc, mybir

BF16 = mybir.dt.bfloat16
F32 = mybir.dt.float32
NPBF16 = ml_dtypes.bfloat16

P = 128  # partitions / k-tile size
QB = 512  # q-block (matmul N; one fp32 PSUM bank)

B, T, C, H, D = 4, 2048, 1024, 16, 64
N_CORES = 8
HL = H // (N_CORES // B)  # heads per core (8)
CL = HL * D  # local head width (512)

# ---------------------------------------------------------------------------
# Per-core Bass program
# ---------------------------------------------------------------------------


def build_kernel(T=T, C=C, HL=HL, D=D, Cout=C):
    CL = HL * D
    n_ct = C // P
    n_mt = CL // P
    n_tt = T // P
    n_qb = T // QB
    n_hp = HL // 2
    dpb = QB // P
    n_cb = Cout // QB
    scale = 1.0 / math.sqrt(D)
    D1 = D + 1
    n_sums = n_hp * n_qb * 2  # one softmax-denominator row per (head, q-block)

    assert C % P == 0 and CL % P == 0 and T % QB == 0 and Cout % QB == 0
    assert HL % 2 == 0 and D == 64 and n_mt == n_hp and n_sums <= P
    assert n_qb == 4 and n_hp == 4 and n_tt == 16  # schedule table below

    nc = bacc.Bacc("TRN2", target_bir_lowering=False, debug=False)
    xT = nc.dram_tensor("xT", [C, T], BF16, kind="ExternalInput")
    wq = nc.dram_tensor("wq", [C, CL], BF16, kind="ExternalInput")
    wk = nc.dram_tensor("wk", [C, CL], BF16, kind="ExternalInput")
    wv = nc.dram_tensor("wv", [C, CL], BF16, kind="ExternalInput")
    wp = nc.dram_tensor("wp", [CL, Cout], BF16, kind="ExternalInput")
    masks = nc.dram_tensor("masks", [P, P], BF16, kind="ExternalInput")
    out = nc.dram_tensor("out", [T, Cout], BF16, kind="ExternalOutput")

    with tile.TileContext(nc) as tc, ExitStack() as ctx:
        persist = ctx.enter_context(tc.tile_pool(name="persist", bufs=1))
        # PSUM budget (8 banks): st 2 x [128,1024] (4) + y 2 x [65,512] (2)
        # + u512 2 x [128,512] (2)
        ps_st = ctx.enter_context(tc.tile_pool(name="ps_st", bufs=2, space="PSUM"))
        ps_y = ctx.enter_context(tc.tile_pool(name="ps_y", bufs=2, space="PSUM"))
        ps_u512 = ctx.enter_context(tc.tile_pool(name="ps_u512", bufs=2, space="PSUM"))
        ppool = ctx.enter_context(tc.tile_pool(name="ppool", bufs=6))
        spool = ctx.enter_context(tc.tile_pool(name="spool", bufs=4))
        bcpool = ctx.enter_context(tc.tile_pool(name="bcpool", bufs=4))
        stage = ctx.enter_context(tc.tile_pool(name="stage", bufs=4))
        dram = ctx.enter_context(tc.tile_pool(name="dram", bufs=1, space="DRAM"))

        # ---- persistent SBUF tensors (consolidated so each input loads
        # with one big DMA instead of one per 128-row tile) ----
        xT_all = persist.tile([P, n_ct * T], BF16, tag="xT", name="xT")
        wv_all = persist.tile([P, n_ct * CL], BF16, tag="wv", name="wv")
        wq_all = persist.tile([P, n_ct * CL], BF16, tag="wq", name="wq")
        wk_all = persist.tile([P, n_ct * CL], BF16, tag="wk", name="wk")
        wp_all = persist.tile([P, n_mt * Cout], BF16, tag="wp", name="wp")
        xT_sb = [xT_all[:, i * T : (i + 1) * T] for i in range(n_ct)]
        wv_sb = [wv_all[:, i * CL : (i + 1) * CL] for i in range(n_ct)]
        wq_sb = [wq_all[:, i * CL : (i + 1) * CL] for i in range(n_ct)]
        wk_sb = [wk_all[:, i * CL : (i + 1) * CL] for i in range(n_ct)]
        wp_sb = [wp_all[:, i * Cout : (i + 1) * Cout] for i in range(n_mt)]
        trimask = persist.tile([P, P], BF16, tag="trimask", name="trimask")
        v1_sb = [
            persist.tile([P, HL * D1], BF16, tag=f"v1_{tt}", name=f"v1_{tt}")
            for tt in range(n_tt)
        ]
        yT_sb = [
            persist.tile([P, T], BF16, tag=f"yT{m}", name=f"yT{m}")
            for m in range(n_mt)
        ]
        q_d = [[None] * n_qb for _ in range(n_hp)]
        k_d = [[None] * n_qb for _ in range(n_hp)]

        sums_d = dram.tile([n_sums, QB], F32, tag="sums_d", name="sums_d")

        # ---- input DMA: one instruction per tensor/chunk (queue-trigger
        # instructions cost ~650ns each, so batching matters). First chunk
        # of xT + wv (for v0..3 and qk(0,0)), then wq/wk, then the rest of
        # xT in two chunks, wp last ----
        T0 = 4 * P  # columns needed by the prologue (v0..3, qk(0,0))
        T1 = 10 * P
        xT_src = xT[:].rearrange("(c p) t -> p c t", p=P)
        xT_dst = xT_all[:].rearrange("p (c t) -> p c t", t=T)
        w_srcs = {
            "wv": wv[:].rearrange("(c p) t -> p c t", p=P),
            "wq": wq[:].rearrange("(c p) t -> p c t", p=P),
            "wk": wk[:].rearrange("(c p) t -> p c t", p=P),
            "wp": wp[:].rearrange("(c p) t -> p c t", p=P),
        }
        w_dsts = {
            "wv": wv_all[:].rearrange("p (c t) -> p c t", t=CL),
            "wq": wq_all[:].rearrange("p (c t) -> p c t", t=CL),
            "wk": wk_all[:].rearrange("p (c t) -> p c t", t=CL),
            "wp": wp_all[:].rearrange("p (c t) -> p c t", t=Cout),
        }
        nc.sync.dma_start(trimask[:], masks[:])
        nc.sync.dma_start(xT_dst[:, :, 0:T0], xT_src[:, :, 0:T0])
        nc.sync.dma_start(w_dsts["wv"], w_srcs["wv"])
        nc.sync.dma_start(w_dsts["wq"], w_srcs["wq"])
        nc.sync.dma_start(w_dsts["wk"], w_srcs["wk"])
        nc.sync.dma_start(xT_dst[:, :, T0:T1], xT_src[:, :, T0:T1])
        nc.sync.dma_start(xT_dst[:, :, T1:T], xT_src[:, :, T1:T])
        nc.sync.dma_start(w_dsts["wp"], w_srcs["wp"])

        def warm_pe(n):
            # junk matmuls on trimask to keep the PE clock-gate (HAM) warm
            # while the PE would otherwise idle (input DMA, final norm).
            ps = ps_u512.tile([P, P], F32, tag="u512", name="warm")
            for _ in range(n):
                nc.tensor.matmul(
                    ps[:, 0:64], trimask[:], trimask[:, 0:64], start=True, stop=True
                )

        warm_pe(64)

        # ---- filler units (one PE burst each, ~0.7-1.7us) ----

        def v_unit(tt):
            t = v1_sb[tt]
            ones_view = t[:].rearrange("p (h e) -> p h e", h=HL)[:, :, D : D + 1]
            nc.vector.memset(ones_view, 1.0)
            ps = ps_u512.tile([P, CL], F32, tag="u512", name="u512")
            for c in range(n_ct):
                nc.tensor.matmul(
                    ps[:],
                    xT_sb[c][:, tt * P : (tt + 1) * P],
                    wv_sb[c][:],
                    start=(c == 0),
                    stop=(c == n_ct - 1),
                )
            dst_view = t[:].rearrange("p (h e) -> p h e", h=HL)[:, :, 0:D]
            src_view = ps[:].rearrange("p (h e) -> p h e", h=HL)
            nc.vector.tensor_copy(dst_view, src_view)

        def qk_unit(hp, b, name):
            w_sb, dst = (wk_sb, k_d) if name == "k" else (wq_sb, q_d)
            t = persist.tile(
                [P, QB], BF16, tag=f"{name}d{hp}_{b}", name=f"{name}d{hp}_{b}"
            )
            dst[hp][b] = t
            ps = ps_u512.tile([P, QB], F32, tag="u512", name="u512")
            for c in range(n_ct):
                nc.tensor.matmul(
                    ps[:],
                    w_sb[c][:, hp * P : (hp + 1) * P],
                    xT_sb[c][:, b * QB : (b + 1) * QB],
                    start=(c == 0),
                    stop=(c == n_ct - 1),
                )
            nc.vector.tensor_copy(t[:], ps[:])

        def proj_unit(tt, cb):
            ps = ps_u512.tile([P, QB], F32, tag="u512", name="u512")
            for m in range(n_mt):
                nc.tensor.matmul(
                    ps[:],
                    yT_sb[m][:, tt * P : (tt + 1) * P],
                    wp_sb[m][:, cb * QB : (cb + 1) * QB],
                    start=(m == 0),
                    stop=(m == n_mt - 1),
                )
            ost = stage.tile([P, QB], BF16, tag="ostage", name="ostage")
            nc.vector.tensor_copy(ost[:], ps[:])
            nc.sync.dma_start(
                out[tt * P : (tt + 1) * P, cb * QB : (cb + 1) * QB], ost[:]
            )

        def sum_row(hp, qb, i):
            return (hp * n_qb + qb) * 2 + i

        def norm_unit(hp, qb):
            # broadcast this q-block's denominators across partitions (the
            # epilogue already parked them in DRAM), reciprocal on the full
            # tile, then scale yT in place.
            bc = bcpool.tile([P, QB], F32, tag="bc", name="bc")
            for i in range(2):
                s = sum_row(hp, qb, i)
                nc.sync.dma_start(
                    bc[64 * i : 64 * i + 64, :],
                    sums_d[s : s + 1, :].to_broadcast((64, QB)),
                )
            rec = bcpool.tile([P, QB], F32, tag="rec", name="rec")
            nc.vector.reciprocal_approx_fast(rec[:], bc[:])
            sl = yT_sb[hp][:, qb * QB : (qb + 1) * QB]
            nc.vector.tensor_mul(sl, sl, rec[:])

        # ---- attention for one (head-pair, q-block): generator that
        # yields once per 2-k-tile group so the scheduler can emit filler
        # PE work into the ACT-paced slack ----

        def attn_qb(hp, qb):
            yts = [
                ps_y.tile([D1, QB], F32, tag="yt", name="yt0"),
                ps_y.tile([D1, QB], F32, tag="yt", name="yt1"),
            ]
            n_kt = dpb * qb + dpb

            def emit_av(kt, pt):
                # diagonal k-tiles only touch q-columns >= P*m
                q0 = P * max(kt - dpb * qb, 0)
                for i in range(2):
                    h = 2 * hp + i
                    nc.tensor.matmul(
                        yts[i][:, q0:QB],
                        v1_sb[kt][:, h * D1 : (h + 1) * D1],
                        pt[:, i * QB + q0 : (i + 1) * QB],
                        start=(kt == 0),
                        stop=(kt == n_kt - 1),
                        skip_group_check=True,
                    )

            pending = []
            for g0 in range(0, n_kt, 2):
                for kt in (g0, g0 + 1):
                    m = kt - dpb * qb  # >=0: diagonal tile index
                    s0 = P * max(m, 0)
                    # combined scores for both heads: [128 k, 1024];
                    # the two matmuls run as concurrent 64x128 row-tiles
                    st = ps_st.tile([P, 2 * QB], F32, tag="st", name="st")
                    for i in range(2):
                        base = 64 * i
                        nc.tensor.matmul(
                            st[:, i * QB + s0 : (i + 1) * QB],
                            k_d[hp][kt // dpb][
                                base : base + 64,
                                (kt % dpb) * P : (kt % dpb + 1) * P,
                            ],
                            q_d[hp][qb][base : base + 64, s0:],
                            start=True,
                            stop=True,
                        )
                    pt = ppool.tile([P, 2 * QB], BF16, tag="pt", name="pt")
                    if m <= 0:
                        nc.scalar.activation(
                            pt[:],
                            st[:],
                            mybir.ActivationFunctionType.Exp,
                            scale=scale,
                        )
                    else:
                        nc.scalar.activation(
                            pt[:].rearrange("p (i q) -> p i q", i=2)[:, :, s0:],
                            st[:].rearrange("p (i q) -> p i q", i=2)[:, :, s0:],
                            mybir.ActivationFunctionType.Exp,
                            scale=scale,
                        )
                    if m >= 0:
                        q0 = P * m
                        sl = pt[:].rearrange("p (i q) -> p i q", i=2)[
                            :, :, q0 : q0 + P
                        ]
                        nc.vector.tensor_mul(
                            sl, sl, trimask[:, None, :].broadcast_to([P, 2, P])
                        )
                    pending.append((kt, pt))
                # AVs lag the scores by one group so the PE never
                # queue-blocks on exp; one mode switch per direction/group.
                while len(pending) > 2:
                    emit_av(*pending.pop(0))
                yield
            for item in pending:
                emit_av(*item)

            # epilogue: one PSUM->SBUF copy per head, then DMA out the
            # unnormalized y (casting SWDGE) and the denominator row
            for i in range(2):
                yt = yts[i]
                ys = spool.tile([D1, QB], F32, tag="ys", name="ys")
                nc.vector.tensor_copy(ys[:], yt[:])
                nc.gpsimd.dma_start(
                    yT_sb[hp][64 * i : 64 * i + 64, qb * QB : (qb + 1) * QB],
                    ys[0:D, :],
                )
                s = sum_row(hp, qb, i)
                nc.sync.dma_start(sums_d[s : s + 1, :], ys[D : D + 1, :])
            yield

        # ---- global schedule ----
        # Filler units per (hp, qb), distributed across that q-block's
        # yield points. Dependencies: scores(hp,qb) need qk(hp,<=qb);
        # AV(kt) needs v(kt); proj(tt of qb) needs all norms for qb.
        fillers = {
            (0, 0): [("qk", 0, 1, "k"), ("qk", 0, 1, "q"), ("v", 4)],
            (0, 1): [("v", 5), ("v", 6), ("v", 7), ("qk", 0, 2, "k"), ("qk", 0, 2, "q")],
            (0, 2): [("v", 8), ("v", 9), ("v", 10), ("v", 11),
                     ("qk", 0, 3, "k"), ("qk", 0, 3, "q")],
            (0, 3): [("v", 12), ("v", 13), ("v", 14), ("v", 15),
                     ("qk", 1, 0, "k"), ("qk", 1, 0, "q")],
            (1, 0): [("qk", 1, 1, "k"), ("qk", 1, 1, "q")],
            (1, 1): [("qk", 1, 2, "k"), ("qk", 1, 2, "q")],
            (1, 2): [("qk", 1, 3, "k"), ("qk", 1, 3, "q")],
            (1, 3): [("qk", 2, 0, "k"), ("qk", 2, 0, "q")],
            (2, 0): [("qk", 2, 1, "k"), ("qk", 2, 1, "q")],
            (2, 1): [("qk", 2, 2, "k"), ("qk", 2, 2, "q")],
            (2, 2): [("qk", 2, 3, "k"), ("qk", 2, 3, "q")],
            (2, 3): [("qk", 3, 0, "k"), ("qk", 3, 0, "q"),
                     ("qk", 3, 1, "k"), ("qk", 3, 1, "q")],
            (3, 0): [("qk", 3, 2, "k"), ("qk", 3, 2, "q"),
                     ("qk", 3, 3, "k"), ("qk", 3, 3, "q")],
            (3, 1): [("proj", tt, cb) for tt in range(0, 4) for cb in range(n_cb)],
            (3, 2): [("proj", tt, cb) for tt in range(4, 8) for cb in range(n_cb)],
            (3, 3): [("proj", tt, cb) for tt in range(8, 12) for cb in range(n_cb)],
        }

        def emit_filler(u):
            if u[0] == "v":
                v_unit(u[1])
            elif u[0] == "qk":
                qk_unit(u[1], u[2], u[3])
            else:
                proj_unit(u[1], u[2])

        # prologue: enough v-tiles and q/k for hp0's first q-block
        for tt in range(4):
            v_unit(tt)
        qk_unit(0, 0, "k")
        qk_unit(0, 0, "q")

        for hp in range(n_hp):
            for qb in range(n_qb):
                gen = attn_qb(hp, qb)
                units = list(fillers[(hp, qb)])
                n_groups = (dpb * qb + dpb) // 2 + 1  # yield points
                # round-robin the units across yield points
                sched = [[] for _ in range(n_groups)]
                for idx, u in enumerate(units):
                    sched[idx % n_groups].append(u)
                gi = 0
                for _ in gen:
                    # norm for the previous q-block right after the first
                    # group of this one (its sums have landed by then)
                    if gi == 0:
                        if qb > 0:
                            norm_unit(hp, qb - 1)
                        elif hp > 0:
                            norm_unit(hp - 1, n_qb - 1)
                    for u in sched[gi]:
                        emit_filler(u)
                    gi += 1

        # tail: last normalization + the last quarter of the projection
        # (junk matmuls keep the PE clock warm across the norm DMA chain)
        norm_unit(n_hp - 1, n_qb - 1)
        warm_pe(24)
        for tt in range(12, 16):
            for cb in range(n_cb):
                proj_unit(tt, cb)

    nc.compile()
    return nc


_PROGRAM_CACHE = {}


def _get_program(C_eff):
    key = C_eff
    if key not in _PROGRAM_CACHE:
        _PROGRAM_CACHE[key] = build_kernel(T=T, C=C_eff, HL=HL, D=D, Cout=C)
    return _PROGRAM_CACHE[key]


def _make_in_maps(x, Wqkv, bqkv):
    """Shard + cast inputs for the 8 cores. Returns (in_maps, C_eff)."""
    if np.any(bqkv):
        # Fold the qkv bias in as an extra contraction row (x gains a ones
        # column), zero-padded up to a multiple of 128.
        C_eff = ((C + 1 + P - 1) // P) * P
        Waug = np.zeros((C_eff, 3 * C), dtype=np.float32)
        Waug[:C] = Wqkv
        Waug[C] = bqkv
    else:
        C_eff = C
        Waug = Wqkv

    masks = (np.arange(P)[:, None] <= np.arange(P)[None, :]).astype(NPBF16)
    in_maps = []
    for core in range(N_CORES):
        b, hh = divmod(core, N_CORES // B)
        xT = np.zeros((C_eff, T), dtype=np.float32)
        xT[:C] = x[b].T
        if C_eff > C:
            xT[C] = 1.0
        c0 = hh * CL
        in_maps.append(
            {
                "xT": xT.astype(NPBF16),
                "wq": np.ascontiguousarray(Waug[:, 0 * C + c0 : 0 * C + c0 + CL]).astype(NPBF16),
                "wk": np.ascontiguousarray(Waug[:, 1 * C + c0 : 1 * C + c0 + CL]).astype(NPBF16),
                "wv": np.ascontiguousarray(Waug[:, 2 * C + c0 : 2 * C + c0 + CL]).astype(NPBF16),
                "wp": None,  # filled below (depends only on hh)
                "masks": masks,
            }
        )
    return in_maps, C_eff


def _run(x, Wqkv, bqkv, Wproj, bproj, trace=False):
    from concourse.bass_utils import run_bass_kernel_spmd

    in_maps, C_eff = _make_in_maps(x, Wqkv, bqkv)
    wp_by_hh = [
        np.ascontiguousarray(Wproj[hh * CL : (hh + 1) * CL, :]).astype(NPBF16)
        for hh in range(N_CORES // B)
    ]
    for core in range(N_CORES):
        in_maps[core]["wp"] = wp_by_hh[core % (N_CORES // B)]

    nc = _get_program(C_eff)
    res = run_bass_kernel_spmd(
        nc, in_maps, core_ids=list(range(N_CORES)), trace=trace
    )

    halves = N_CORES // B
    y = np.empty((B, T, C), dtype=np.float32)
    for b in range(B):
        acc = res.results[b * halves]["out"].astype(np.float32)
        for hh in range(1, halves):
            acc = acc + res.results[b * halves + hh]["out"].astype(np.float32)
        y[b] = acc + bproj.astype(np.float32)
    return y, res


def kernel(x, Wqkv, bqkv, Wproj, bproj):
    y, _ = _run(
        np.asarray(x, dtype=np.float32),
        np.asarray(Wqkv, dtype=np.float32),
        np.asarray(bqkv, dtype=np.float32),
        np.asarray(Wproj, dtype=np.float32),
        np.asarray(bproj, dtype=np.float32),
        trace=False,
    )
    return y


# revision 24
# speedup vs baseline: 1.0198x; 1.0027x over previous
"""Causal self-attention on 8 TRN2 NeuronCores (Bass/Tile, SPMD).

Problem: y = CausalSelfAttention(x; Wqkv, bqkv, Wproj, bproj)
  x [B=4, T=2048, C=1024], H=16 heads, D=64.

Sharding: core c = (batch b = c//2, head-half hh = c%2). Each core computes
q/k/v for its 8 heads of its batch (Wqkv column-sharded), full causal
attention for those heads, and a partial output projection (Wproj
row-sharded). Host sums the two bf16 partials per batch and adds bproj.

Per-core kernel (all matmuls bf16 with fp32 PSUM accumulation):
  - q,k are produced d-major ([CL, T]) so QK^T needs no transposes;
    scores come out k-major [128 k, 512 q] per tile. The two heads of a
    pair run as concurrent 64x128 row-tiles of the PE array.
  - softmax skips the max-subtraction (scores are O(1); exp is safe),
    one fused exp on the Scalar engine per k-tile; causal mask is a bf16
    multiply on the diagonal blocks only. Row sums come free from an
    extra ones-column appended to each per-head V tile (M=65 AV matmul);
    1/sum is broadcast across partitions via a DRAM round-trip DMA.
  - Sub-diagonal k-tiles are skipped entirely (half the attention work).

Scheduling (the attention exp stream saturates the Scalar engine while
the PE has slack; everything else is interleaved into that slack):
  - v-projection tiles, q/k projections for later head-pairs, softmax
    normalization, and 3/4 of the output projection are emitted as
    fine-grained filler units between attention k-tile groups, so the
    PE and ACT engines both stay busy from prologue to tail.
  - k-tiles are processed in groups of two (scores+exp for both, then
    the four staggered AV matmuls) to halve PE tiling-mode switches.
  - normalization runs per (head-pair, q-block) as soon as that
    q-block's denominators land; the output projection for a T-slice
    starts as soon as the last head-pair has normalized it.
"""

import math
from contextlib import ExitStack

import numpy as np
import ml_dtypes

import concourse.tile as tile
from concourse import bacc, mybir

BF16 = mybir.dt.bfloat16
F32 = mybir.dt.float32
NPBF16 = ml_dtypes.bfloat16

P = 128  # partitions / k-tile size
QB = 512  # q-block (matmul N; one fp32 PSUM bank)

B, T, C, H, D = 4, 2048, 1024, 16, 64
N_CORES = 8
HL = H // (N_CORES // B)  # heads per core (8)
CL = HL * D  # local head width (512)

# ---------------------------------------------------------------------------
# Per-core Bass program
# ---------------------------------------------------------------------------


def build_kernel(T=T, C=C, HL=HL, D=D, Cout=C):
    CL = HL * D
    n_ct = C // P
    n_mt = CL // P
    n_tt = T // P
    n_qb = T // QB
    n_hp = HL // 2
    dpb = QB // P
    n_cb = Cout // QB
    scale = 1.0 / math.sqrt(D)
    D1 = D + 1
    n_sums = n_hp * n_qb * 2  # one softmax-denominator row per (head, q-block)

    assert C % P == 0 and CL % P == 0 and T % QB == 0 and Cout % QB == 0
    assert HL % 2 == 0 and D == 64 and n_mt == n_hp and n_sums <= P
    assert n_qb == 4 and n_hp == 4 and n_tt == 16  # schedule table below

    nc = bacc.Bacc("TRN2", target_bir_lowering=False, debug=False)
    xT = nc.dram_tensor("xT", [C, T], BF16, kind="ExternalInput")
    wq = nc.dram_tensor("wq", [C, CL], BF16, kind="ExternalInput")
    wk = nc.dram_tensor("wk", [C, CL], BF16, kind="ExternalInput")
    wv = nc.dram_tensor("wv", [C, CL], BF16, kind="ExternalInput")
    wp = nc.dram_tensor("wp", [CL, Cout], BF16, kind="ExternalInput")
    masks = nc.dram_tensor("masks", [P, P], BF16, kind="ExternalInput")
    out = nc.dram_tensor("out", [T, Cout], BF16, kind="ExternalOutput")

    with tile.TileContext(nc) as tc, ExitStack() as ctx:
        persist = ctx.enter_context(tc.tile_pool(name="persist", bufs=1))
        # PSUM budget (8 banks): st 2 x [128,1024] (4) + y 2 x [65,512] (2)
        # + u512 2 x [128,512] (2)
        ps_st = ctx.enter_context(tc.tile_pool(name="ps_st", bufs=2, space="PSUM"))
        ps_y = ctx.enter_context(tc.tile_pool(name="ps_y", bufs=2, space="PSUM"))
        ps_u512 = ctx.enter_context(tc.tile_pool(name="ps_u512", bufs=2, space="PSUM"))
        ppool = ctx.enter_context(tc.tile_pool(name="ppool", bufs=6))
        spool = ctx.enter_context(tc.tile_pool(name="spool", bufs=4))
        bcpool = ctx.enter_context(tc.tile_pool(name="bcpool", bufs=4))
        stage = ctx.enter_context(tc.tile_pool(name="stage", bufs=4))
        dram = ctx.enter_context(tc.tile_pool(name="dram", bufs=1, space="DRAM"))

        # ---- persistent SBUF tensors (consolidated so each input loads
        # with one big DMA instead of one per 128-row tile) ----
        xT_all = persist.tile([P, n_ct * T], BF16, tag="xT", name="xT")
        wv_all = persist.tile([P, n_ct * CL], BF16, tag="wv", name="wv")
        wq_all = persist.tile([P, n_ct * CL], BF16, tag="wq", name="wq")
        wk_all = persist.tile([P, n_ct * CL], BF16, tag="wk", name="wk")
        wp_all = persist.tile([P, n_mt * Cout], BF16, tag="wp", name="wp")
        xT_sb = [xT_all[:, i * T : (i + 1) * T] for i in range(n_ct)]
        wv_sb = [wv_all[:, i * CL : (i + 1) * CL] for i in range(n_ct)]
        wq_sb = [wq_all[:, i * CL : (i + 1) * CL] for i in range(n_ct)]
        wk_sb = [wk_all[:, i * CL : (i + 1) * CL] for i in range(n_ct)]
        wp_sb = [wp_all[:, i * Cout : (i + 1) * Cout] for i in range(n_mt)]
        trimask = persist.tile([P, P], BF16, tag="trimask", name="trimask")
        v1_sb = [
            persist.tile([P, HL * D1], BF16, tag=f"v1_{tt}", name=f"v1_{tt}")
            for tt in range(n_tt)
        ]
        yT_sb = [
            persist.tile([P, T], BF16, tag=f"yT{m}", name=f"yT{m}")
            for m in range(n_mt)
        ]
        q_d = [[None] * n_qb for _ in range(n_hp)]
        k_d = [[None] * n_qb for _ in range(n_hp)]

        sums_d = dram.tile([n_sums, QB], F32, tag="sums_d", name="sums_d")

        # ---- input DMA: one instruction per tensor/chunk (queue-trigger
        # instructions cost ~650ns each, so batching matters). First chunk
        # of xT + wv (for v0..3 and qk(0,0)), then wq/wk, then the rest of
        # xT in two chunks, wp last ----
        T0 = 4 * P  # columns needed by the prologue (v0..3, qk(0,0))
        T1 = 10 * P
        xT_src = xT[:].rearrange("(c p) t -> p c t", p=P)
        xT_dst = xT_all[:].rearrange("p (c t) -> p c t", t=T)
        w_srcs = {
            "wv": wv[:].rearrange("(c p) t -> p c t", p=P),
            "wq": wq[:].rearrange("(c p) t -> p c t", p=P),
            "wk": wk[:].rearrange("(c p) t -> p c t", p=P),
            "wp": wp[:].rearrange("(c p) t -> p c t", p=P),
        }
        w_dsts = {
            "wv": wv_all[:].rearrange("p (c t) -> p c t", t=CL),
            "wq": wq_all[:].rearrange("p (c t) -> p c t", t=CL),
            "wk": wk_all[:].rearrange("p (c t) -> p c t", t=CL),
            "wp": wp_all[:].rearrange("p (c t) -> p c t", t=Cout),
        }
        nc.sync.dma_start(trimask[:], masks[:])
        nc.sync.dma_start(xT_dst[:, :, 0:T0], xT_src[:, :, 0:T0])
        nc.sync.dma_start(w_dsts["wv"], w_srcs["wv"])
        nc.sync.dma_start(w_dsts["wq"], w_srcs["wq"])
        nc.sync.dma_start(w_dsts["wk"], w_srcs["wk"])
        nc.sync.dma_start(xT_dst[:, :, T0:T1], xT_src[:, :, T0:T1])
        nc.sync.dma_start(xT_dst[:, :, T1:T], xT_src[:, :, T1:T])
        nc.sync.dma_start(w_dsts["wp"], w_srcs["wp"])

        def warm_pe(n):
            # junk matmuls on trimask to keep the PE clock-gate (HAM) warm
            # while the PE would otherwise idle (input DMA, final norm).
            ps = ps_u512.tile([P, P], F32, tag="u512", name="warm")
            for _ in range(n):
                nc.tensor.matmul(
                    ps[:, 0:64], trimask[:], trimask[:, 0:64], start=True, stop=True
                )

        warm_pe(135)

        # ---- filler units (one PE burst each, ~0.7-1.7us) ----

        def v_unit(tt):
            t = v1_sb[tt]
            ones_view = t[:].rearrange("p (h e) -> p h e", h=HL)[:, :, D : D + 1]
            nc.vector.memset(ones_view, 1.0)
            ps = ps_u512.tile([P, CL], F32, tag="u512", name="u512")
            for c in range(n_ct):
                nc.tensor.matmul(
                    ps[:],
                    xT_sb[c][:, tt * P : (tt + 1) * P],
                    wv_sb[c][:],
                    start=(c == 0),
                    stop=(c == n_ct - 1),
                )
            dst_view = t[:].rearrange("p (h e) -> p h e", h=HL)[:, :, 0:D]
            src_view = ps[:].rearrange("p (h e) -> p h e", h=HL)
            nc.vector.tensor_copy(dst_view, src_view)

        def qk_unit(hp, b, name):
            w_sb, dst = (wk_sb, k_d) if name == "k" else (wq_sb, q_d)
            t = persist.tile(
                [P, QB], BF16, tag=f"{name}d{hp}_{b}", name=f"{name}d{hp}_{b}"
            )
            dst[hp][b] = t
            ps = ps_u512.tile([P, QB], F32, tag="u512", name="u512")
            for c in range(n_ct):
                nc.tensor.matmul(
                    ps[:],
                    w_sb[c][:, hp * P : (hp + 1) * P],
                    xT_sb[c][:, b * QB : (b + 1) * QB],
                    start=(c == 0),
                    stop=(c == n_ct - 1),
                )
            nc.vector.tensor_copy(t[:], ps[:])

        def proj_unit(tt, cb):
            ps = ps_u512.tile([P, QB], F32, tag="u512", name="u512")
            for m in range(n_mt):
                nc.tensor.matmul(
                    ps[:],
                    yT_sb[m][:, tt * P : (tt + 1) * P],
                    wp_sb[m][:, cb * QB : (cb + 1) * QB],
                    start=(m == 0),
                    stop=(m == n_mt - 1),
                )
            ost = stage.tile([P, QB], BF16, tag="ostage", name="ostage")
            nc.vector.tensor_copy(ost[:], ps[:])
            nc.sync.dma_start(
                out[tt * P : (tt + 1) * P, cb * QB : (cb + 1) * QB], ost[:]
            )

        def sum_row(hp, qb, i):
            return (hp * n_qb + qb) * 2 + i

        def norm_unit(hp, qb):
            # broadcast this q-block's denominators across partitions (the
            # epilogue already parked them in DRAM; one DMA queue per head
            # so the two run concurrently), then divide yT in place.
            bc = bcpool.tile([P, QB], F32, tag="bc", name="bc")
            for i, q in ((0, nc.sync), (1, nc.gpsimd)):
                s = sum_row(hp, qb, i)
                q.dma_start(
                    bc[64 * i : 64 * i + 64, :],
                    sums_d[s : s + 1, :].to_broadcast((64, QB)),
                )
            rec = bcpool.tile([P, QB], F32, tag="rec", name="rec")
            nc.vector.reciprocal_approx_fast(rec[:], bc[:])
            sl = yT_sb[hp][:, qb * QB : (qb + 1) * QB]
            nc.vector.tensor_mul(sl, sl, rec[:])

        # ---- attention for one (head-pair, q-block): generator that
        # yields once per 2-k-tile group so the scheduler can emit filler
        # PE work into the ACT-paced slack ----

        def attn_qb(hp, qb):
            yts = [
                ps_y.tile([D1, QB], F32, tag="yt", name="yt0"),
                ps_y.tile([D1, QB], F32, tag="yt", name="yt1"),
            ]
            n_kt = dpb * qb + dpb

            def emit_av(kt, pt):
                # diagonal k-tiles only touch q-columns >= P*m
                q0 = P * max(kt - dpb * qb, 0)
                for i in range(2):
                    h = 2 * hp + i
                    nc.tensor.matmul(
                        yts[i][:, q0:QB],
                        v1_sb[kt][:, h * D1 : (h + 1) * D1],
                        pt[:, i * QB + q0 : (i + 1) * QB],
                        start=(kt == 0),
                        stop=(kt == n_kt - 1),
                        skip_group_check=True,
                    )

            pending = []
            for g0 in range(0, n_kt, 2):
                for kt in (g0, g0 + 1):
                    m = kt - dpb * qb  # >=0: diagonal tile index
                    s0 = P * max(m, 0)
                    # combined scores for both heads: [128 k, 1024];
                    # the two matmuls run as concurrent 64x128 row-tiles
                    st = ps_st.tile([P, 2 * QB], F32, tag="st", name="st")
                    for i in range(2):
                        base = 64 * i
                        nc.tensor.matmul(
                            st[:, i * QB + s0 : (i + 1) * QB],
                            k_d[hp][kt // dpb][
                                base : base + 64,
                                (kt % dpb) * P : (kt % dpb + 1) * P,
                            ],
                            q_d[hp][qb][base : base + 64, s0:],
                            start=True,
                            stop=True,
                        )
                    pt = ppool.tile([P, 2 * QB], BF16, tag="pt", name="pt")
                    if m <= 0:
                        nc.scalar.activation(
                            pt[:],
                            st[:],
                            mybir.ActivationFunctionType.Exp,
                            scale=scale,
                        )
                    else:
                        nc.scalar.activation(
                            pt[:].rearrange("p (i q) -> p i q", i=2)[:, :, s0:],
                            st[:].rearrange("p (i q) -> p i q", i=2)[:, :, s0:],
                            mybir.ActivationFunctionType.Exp,
                            scale=scale,
                        )
                    if m >= 0:
                        q0 = P * m
                        sl = pt[:].rearrange("p (i q) -> p i q", i=2)[
                            :, :, q0 : q0 + P
                        ]
                        nc.vector.tensor_mul(
                            sl, sl, trimask[:, None, :].broadcast_to([P, 2, P])
                        )
                    pending.append((kt, pt))
                # AVs lag the scores by one group so the PE never
                # queue-blocks on exp; one mode switch per direction/group.
                while len(pending) > 2:
                    emit_av(*pending.pop(0))
                yield
            for item in pending:
                emit_av(*item)

            # epilogue: one PSUM->SBUF copy per head, then DMA out the
            # unnormalized y (casting SWDGE) and the denominator row
            for i in range(2):
                yt = yts[i]
                ys = spool.tile([D1, QB], F32, tag="ys", name="ys")
                nc.vector.tensor_copy(ys[:], yt[:])
                nc.gpsimd.dma_start(
                    yT_sb[hp][64 * i : 64 * i + 64, qb * QB : (qb + 1) * QB],
                    ys[0:D, :],
                )
                s = sum_row(hp, qb, i)
                nc.sync.dma_start(sums_d[s : s + 1, :], ys[D : D + 1, :])
            yield

        # ---- global schedule ----
        # Filler units per (hp, qb), distributed across that q-block's
        # yield points. Dependencies: scores(hp,qb) need qk(hp,<=qb);
        # AV(kt) needs v(kt); proj(tt of qb) needs all norms for qb.
        fillers = {
            (0, 0): [("qk", 0, 1, "k"), ("qk", 0, 1, "q"), ("v", 4)],
            (0, 1): [("v", 5), ("v", 6), ("v", 7), ("qk", 0, 2, "k"), ("qk", 0, 2, "q")],
            (0, 2): [("v", 8), ("v", 9), ("v", 10), ("v", 11),
                     ("qk", 0, 3, "k"), ("qk", 0, 3, "q")],
            (0, 3): [("v", 12), ("v", 13), ("v", 14), ("v", 15),
                     ("qk", 1, 0, "k"), ("qk", 1, 0, "q")],
            (1, 0): [("qk", 1, 1, "k"), ("qk", 1, 1, "q")],
            (1, 1): [("qk", 1, 2, "k"), ("qk", 1, 2, "q")],
            (1, 2): [("qk", 1, 3, "k"), ("qk", 1, 3, "q")],
            (1, 3): [("qk", 2, 0, "k"), ("qk", 2, 0, "q")],
            (2, 0): [("qk", 2, 1, "k"), ("qk", 2, 1, "q")],
            (2, 1): [("qk", 2, 2, "k"), ("qk", 2, 2, "q")],
            (2, 2): [("qk", 2, 3, "k"), ("qk", 2, 3, "q")],
            (2, 3): [("qk", 3, 0, "k"), ("qk", 3, 0, "q"),
                     ("qk", 3, 1, "k"), ("qk", 3, 1, "q")],
            (3, 0): [("qk", 3, 2, "k"), ("qk", 3, 2, "q"),
                     ("qk", 3, 3, "k"), ("qk", 3, 3, "q")],
            (3, 1): [("proj", tt, cb) for tt in range(0, 4) for cb in range(n_cb)],
            (3, 2): [("proj", tt, cb) for tt in range(4, 8) for cb in range(n_cb)],
            (3, 3): [("proj", tt, cb) for tt in range(8, 12) for cb in range(n_cb)],
        }

        def emit_filler(u):
            if u[0] == "v":
                v_unit(u[1])
            elif u[0] == "qk":
                qk_unit(u[1], u[2], u[3])
            else:
                proj_unit(u[1], u[2])

        # prologue: enough v-tiles and q/k for hp0's first q-block
        for tt in range(4):
            v_unit(tt)
        qk_unit(0, 0, "k")
        qk_unit(0, 0, "q")

        for hp in range(n_hp):
            for qb in range(n_qb):
                gen = attn_qb(hp, qb)
                units = list(fillers[(hp, qb)])
                n_groups = (dpb * qb + dpb) // 2 + 1  # yield points
                # round-robin the units across yield points
                sched = [[] for _ in range(n_groups)]
                for idx, u in enumerate(units):
                    sched[idx % n_groups].append(u)
                gi = 0
                for _ in gen:
                    # norm for the previous q-block right after the first
                    # group of this one (its sums have landed by then)
                    if gi == 0:
                        if qb > 0:
                            norm_unit(hp, qb - 1)
                        elif hp > 0:
                            norm_unit(hp - 1, n_qb - 1)
                    for u in sched[gi]:
                        emit_filler(u)
                    gi += 1

        # tail: last normalization + the last quarter of the projection
        # (junk matmuls keep the PE clock warm across the norm DMA chain)
        norm_unit(n_hp - 1, n_qb - 1)
        warm_pe(110)
        for tt in range(12, 16):
            for cb in range(n_cb):
                proj_unit(tt, cb)

    nc.compile()
    return nc


_PROGRAM_CACHE = {}


def _get_program(C_eff):
    key = C_eff
    if key not in _PROGRAM_CACHE:
        _PROGRAM_CACHE[key] = build_kernel(T=T, C=C_eff, HL=HL, D=D, Cout=C)
    return _PROGRAM_CACHE[key]


def _make_in_maps(x, Wqkv, bqkv):
    """Shard + cast inputs for the 8 cores. Returns (in_maps, C_eff)."""
    if np.any(bqkv):
        # Fold the qkv bias in as an extra contraction row (x gains a ones
        # column), zero-padded up to a multiple of 128.
        C_eff = ((C + 1 + P - 1) // P) * P
        Waug = np.zeros((C_eff, 3 * C), dtype=np.float32)
        Waug[:C] = Wqkv
        Waug[C] = bqkv
    else:
        C_eff = C
        Waug = Wqkv

    masks = (np.arange(P)[:, None] <= np.arange(P)[None, :]).astype(NPBF16)
    in_maps = []
    for core in range(N_CORES):
        b, hh = divmod(core, N_CORES // B)
        xT = np.zeros((C_eff, T), dtype=np.float32)
        xT[:C] = x[b].T
        if C_eff > C:
            xT[C] = 1.0
        c0 = hh * CL
        in_maps.append(
            {
                "xT": xT.astype(NPBF16),
                "wq": np.ascontiguousarray(Waug[:, 0 * C + c0 : 0 * C + c0 + CL]).astype(NPBF16),
                "wk": np.ascontiguousarray(Waug[:, 1 * C + c0 : 1 * C + c0 + CL]).astype(NPBF16),
                "wv": np.ascontiguousarray(Waug[:, 2 * C + c0 : 2 * C + c0 + CL]).astype(NPBF16),
                "wp": None,  # filled below (depends only on hh)
                "masks": masks,
            }
        )
    return in_maps, C_eff


def _run(x, Wqkv, bqkv, Wproj, bproj, trace=False):
    from concourse.bass_utils import run_bass_kernel_spmd

    in_maps, C_eff = _make_in_maps(x, Wqkv, bqkv)
    wp_by_hh = [
        np.ascontiguousarray(Wproj[hh * CL : (hh + 1) * CL, :]).astype(NPBF16)
        for hh in range(N_CORES // B)
    ]
    for core in range(N_CORES):
        in_maps[core]["wp"] = wp_by_hh[core % (N_CORES // B)]

    nc = _get_program(C_eff)
    res = run_bass_kernel_spmd(
        nc, in_maps, core_ids=list(range(N_CORES)), trace=trace
    )

    halves = N_CORES // B
    y = np.empty((B, T, C), dtype=np.float32)
    for b in range(B):
        acc = res.results[b * halves]["out"].astype(np.float32)
        for hh in range(1, halves):
            acc = acc + res.results[b * halves + hh]["out"].astype(np.float32)
        y[b] = acc + bproj.astype(np.float32)
    return y, res


def kernel(x, Wqkv, bqkv, Wproj, bproj):
    y, _ = _run(
        np.asarray(x, dtype=np.float32),
        np.asarray(Wqkv, dtype=np.float32),
        np.asarray(bqkv, dtype=np.float32),
        np.asarray(Wproj, dtype=np.float32),
        np.asarray(bproj, dtype=np.float32),
        trace=False,
    )
    return y
